# revision 31
# baseline (speedup 1.0000x reference)
"""2-layer GraphSAGE (PyG SAGEConv, project=True, mean agg) on 8 trn2 NeuronCores.

Strategy (graph/data parallel, hardcoded for N=50000, E=800000, D=128, 8 cores):
  - Nodes sharded by contiguous ranges of 6250 (padded to 6272 = 49*128) per core.
  - Host preprocesses edges: sorted by (dst core, dst block, src half, src),
    padded so every (block, half) has a uniform chunk count across cores (SPMD).
  - Device per layer:
      * project own rows: p = relu(x @ WpT + bp) -> fp16 (x fed pre-transposed
        from host), AllGather into a replicated [50176,128] fp16 table in DRAM.
      * dma_gather (SWDGE, single_packet=True: one 256B descriptor per row)
        fetches message rows group-by-group from the two int16-indexed halves.
      * scatter via one-hot matmuls: aggT[d,k] += msg[e,d]^T @ onehot[e,k];
        the KT one-hots of a block are built in ONE DVE is_equal using 3-D
        broadcast APs against an iota tile.
      * mean via per-dst invdeg multiply, then output matmuls + bias (+relu).
      * layer-2 projection is fused into the layer-1 aggregate block loop.
  - Layer-2 output rows are written per core and concatenated on host.
"""

import math
from contextlib import ExitStack

import numpy as np

import concourse.bacc as bacc
import concourse.bass as bass
import concourse.tile as tile
from concourse import library_config, mybir
from concourse.bass_utils import run_bass_kernel_spmd

P = 128
D = 128
CORES = 8
N_NODES = 50000
N_EDGES = 800000

AF = mybir.ActivationFunctionType
OP = mybir.AluOpType
dt = mybir.dt

DEBUG_OUTPUTS = False
SINGLE_PACKET = False


def _plan(n_nodes, cores):
    nloc = n_nodes // cores
    assert nloc * cores == n_nodes
    nb = math.ceil(nloc / P)
    nloc_pad = nb * P
    npad = cores * nloc_pad
    nhalf = npad // 2
    assert nhalf < 32768, "dma_gather idx is int16"
    return nloc, nb, nloc_pad, npad, nhalf


def preprocess(edge_index, n_nodes, cores):
    """Returns per-core gather/scatter metadata + uniform chunk counts K0, K1."""
    nloc, nb, nloc_pad, npad, nhalf = _plan(n_nodes, cores)
    src = np.asarray(edge_index[0], dtype=np.int64)
    dst = np.asarray(edge_index[1], dtype=np.int64)
    E = src.shape[0]

    deg = np.bincount(dst, minlength=n_nodes).astype(np.float64)
    invdeg = (1.0 / np.maximum(deg, 1.0)).astype(np.float32)

    csrc = src // nloc
    r_src = csrc * nloc_pad + (src - csrc * nloc)  # padded row id of source
    half = (r_src >= nhalf).astype(np.int64)
    idx_in_half = (r_src - half * nhalf).astype(np.int64)

    cdst = dst // nloc
    ldst = dst - cdst * nloc
    blk = ldst // P
    dblk = ldst % P

    # sort edges by (dst core, dst block, src half, src row) — src order gives
    # the DMA engines ascending-address locality within each gather list
    order = np.lexsort((idx_in_half, half, blk, cdst))
    s_half = half[order]
    s_idx = idx_in_half[order]
    s_dblk = dblk[order]
    key = ((cdst[order] * nb + blk[order]) * 2 + s_half).astype(np.int64)

    counts = np.bincount(key, minlength=cores * nb * 2)
    starts = np.zeros(cores * nb * 2 + 1, dtype=np.int64)
    np.cumsum(counts, out=starts[1:])
    rank = np.arange(E, dtype=np.int64) - starts[key]

    cnt = counts.reshape(cores, nb, 2)
    K0 = max(1, int(math.ceil(cnt[:, :, 0].max() / P)))
    K1 = max(1, int(math.ceil(cnt[:, :, 1].max() / P)))

    # idx arrays: [cores, nb, K*P] int16 (pad = 0, harmless row gathered,
    # neutralized by dloc pad = 255 in the one-hot); dloc: [cores, nb, (K0+K1)*P]
    idx0 = np.zeros((cores, nb, K0 * P), dtype=np.int16)
    idx1 = np.zeros((cores, nb, K1 * P), dtype=np.int16)
    dloc = np.full((cores, nb, (K0 + K1) * P), 255.0, dtype=np.float16)

    core_k = key // (nb * 2)
    blk_k = (key // 2) % nb
    m0 = s_half == 0
    m1 = ~m0
    idx0[core_k[m0], blk_k[m0], rank[m0]] = s_idx[m0].astype(np.int16)
    idx1[core_k[m1], blk_k[m1], rank[m1]] = s_idx[m1].astype(np.int16)
    dloc[core_k[m0], blk_k[m0], rank[m0]] = s_dblk[m0].astype(np.float16)
    dloc[core_k[m1], blk_k[m1], K0 * P + rank[m1]] = s_dblk[m1].astype(np.float16)

    def wrap_idx(a):  # [nb, K*P] -> [128, nb*K*P//16] dma_gather layout
        flat = a.reshape(-1)
        w = flat.reshape(-1, 16).T  # [16, I/16]
        return np.tile(w, (8, 1)).copy()

    per_core = []
    for c in range(cores):
        # dloc layout [P, nb*KT]: column (b*KT + t), row = edge lane
        dl = np.ascontiguousarray(dloc[c].reshape(nb * (K0 + K1), P).T)
        inv = np.ones(nloc_pad, dtype=np.float32)
        inv[:nloc] = invdeg[c * nloc : (c + 1) * nloc]
        per_core.append(
            dict(
                idx0=wrap_idx(idx0[c]),
                idx1=wrap_idx(idx1[c]),
                dloc=dl,
                invd=np.broadcast_to(inv[None, :], (P, nloc_pad)).copy(),
            )
        )
    return per_core, K0, K1


def build_nc(n_nodes, cores, K0, K1, G):
    nloc, nb, nloc_pad, npad, nhalf = _plan(n_nodes, cores)
    assert nb % G == 0
    ngroups = nb // G
    KT = K0 + K1

    nc = bacc.Bacc("TRN2", target_bir_lowering=False, debug=False, num_devices=cores,
                   num_swdge_queues=4)

    xT_d = nc.dram_tensor("xT", [P, nloc_pad], dt.float16, kind="ExternalInput").ap()
    idx0_d = nc.dram_tensor("idx0", [P, nb * K0 * P // 16], dt.int16, kind="ExternalInput").ap()
    idx1_d = nc.dram_tensor("idx1", [P, nb * K1 * P // 16], dt.int16, kind="ExternalInput").ap()
    dloc_d = nc.dram_tensor("dloc", [P, nb * KT], dt.float16, kind="ExternalInput").ap()
    invd_d = nc.dram_tensor("invd", [P, nloc_pad], dt.float32, kind="ExternalInput").ap()
    wdram = {
        n: nc.dram_tensor(n, [P, D], dt.float16, kind="ExternalInput").ap()
        for n in ["Wp1T", "Wl1T", "Wr1T", "Wp2T", "Wl2T", "Wr2T"]
    }
    bp1b_d = nc.dram_tensor("bp1b", [P, D], dt.float32, kind="ExternalInput").ap()
    bl1c_d = nc.dram_tensor("bl1c", [P, 1], dt.float32, kind="ExternalInput").ap()
    bp2b_d = nc.dram_tensor("bp2b", [P, D], dt.float32, kind="ExternalInput").ap()
    bl2b_d = nc.dram_tensor("bl2b", [P, D], dt.float32, kind="ExternalInput").ap()
    iota_d = nc.dram_tensor("iota", [P, P], dt.float16, kind="ExternalInput").ap()

    out_own = nc.dram_tensor("out_own", [nloc_pad, D], dt.float32, kind="ExternalOutput").ap()
    h1own = nc.dram_tensor("h1own", [nloc_pad, D], dt.float16).ap()
    h2own = nc.dram_tensor("h2own", [nloc_pad, D], dt.float16).ap()
    table1 = nc.dram_tensor("table1", [npad, D], dt.float16, addr_space="Shared").ap()
    table2 = nc.dram_tensor("table2", [npad, D], dt.float16, addr_space="Shared").ap()
    if DEBUG_OUTPUTS:
        dbg1 = nc.dram_tensor("dbg1", [nloc_pad, D], dt.float16, kind="ExternalOutput").ap()
        dbg2 = nc.dram_tensor("dbg2", [nloc_pad, D], dt.float16, kind="ExternalOutput").ap()

    groups_all = [list(range(cores))]

    with tile.TileContext(nc) as tc, ExitStack() as ctx:
        const = ctx.enter_context(tc.tile_pool(name="const", bufs=1))
        persist = ctx.enter_context(tc.tile_pool(name="persist", bufs=1))
        stage_p = ctx.enter_context(tc.tile_pool(name="stage", bufs=2))
        ohp = ctx.enter_context(tc.tile_pool(name="oh", bufs=4))
        aggp = ctx.enter_context(tc.tile_pool(name="aggp", bufs=4))
        work = ctx.enter_context(tc.tile_pool(name="work", bufs=6))
        outp = ctx.enter_context(tc.tile_pool(name="outp", bufs=6))
        psum_agg = ctx.enter_context(tc.tile_pool(name="psum_agg", bufs=3, space="PSUM"))
        psum_mm = ctx.enter_context(tc.tile_pool(name="psum_mm", bufs=5, space="PSUM"))

        nc.gpsimd.load_library(library_config.mlp)

        def cload(ap_dram, shape, dtype, tag):
            t = const.tile(shape, dtype, tag=tag)
            nc.sync.dma_start(t[:], ap_dram)
            return t

        wsb = {n: cload(wdram[n][:, :], [P, D], dt.float16, n) for n in wdram}
        bp1b = cload(bp1b_d[:, :], [P, D], dt.float32, "bp1b")
        bl1c = cload(bl1c_d[:, :], [P, 1], dt.float32, "bl1c")
        bp2b = cload(bp2b_d[:, :], [P, D], dt.float32, "bp2b")
        bl2b = cload(bl2b_d[:, :], [P, D], dt.float32, "bl2b")
        iota = cload(iota_d[:, :], [P, P], dt.float16, "iota")
        dloc_sb = cload(dloc_d[:, :], [P, nb * KT], dt.float16, "dloc")
        invd_sb = cload(invd_d[:, :], [P, nloc_pad], dt.float32, "invd")
        idx0_sb = cload(idx0_d[:, :], [P, nb * K0 * P // 16], dt.int16, "idx0")
        idx1_sb = cload(idx1_d[:, :], [P, nb * K1 * P // 16], dt.int16, "idx1")

        xT_sb = persist.tile([P, nloc_pad], dt.float16, tag="xT")
        nc.sync.dma_start(xT_sb[:], xT_d[:, :])
        h1T_sb = persist.tile([P, nloc_pad], dt.float16, tag="h1T")

        def proj_block(src_sb, sl, WT, bias_full, dst_dram):
            # dst_dram[sl] = relu(src^T @ W^T + b), node-major fp16 rows
            p_ps = psum_mm.tile([P, D], dt.float32, tag="mm")
            nc.tensor.matmul(p_ps[:], lhsT=src_sb[:, sl], rhs=WT[:], start=True, stop=True)
            pb = work.tile([P, D], dt.float32, tag="pb")
            nc.vector.tensor_tensor(out=pb[:], in0=p_ps[:], in1=bias_full[:], op=OP.add)
            pr = outp.tile([P, D], dt.float16, tag="pr")
            nc.scalar.activation(pr[:], pb[:], AF.Relu)
            nc.sync.dma_start(dst_dram[sl, :], pr[:])

        # ---------------- Phase A: layer-1 projection of own rows ----------
        for b in range(nb):
            proj_block(xT_sb, slice(b * P, (b + 1) * P), wsb["Wp1T"], bp1b, h1own)

        nc.gpsimd.collective_compute(
            "AllGather", OP.bypass, replica_groups=groups_all,
            ins=[h1own[:, :]], outs=[table1[:, :]],
        )

        # ---------------- message+aggregate for one layer -------------------
        # sub-gather width in 128-row chunks (sixth of a group per gather;
        # parts of both table-halves round-robin over all four SWDGE queues)
        SUB = (G * max(K0, K1) + 5) // 6

        def subgather(table_half, idx_sb, g, K, tagbase, qn):
            nchunks = G * K
            parts = []
            for pi, j in enumerate(range(0, nchunks, SUB)):
                w = min(SUB, nchunks - j)
                st = stage_p.tile([P, w, D], dt.float16, tag=f"{tagbase}_{j}")
                cstart = (g * nchunks + j) * P // 16
                nc.gpsimd.dma_gather(
                    st[:], table_half, idx_sb[:, cstart : cstart + w * P // 16],
                    w * P, w * P, D, single_packet=SINGLE_PACKET,
                    queue_num=(qn + pi) % 4,
                )
                parts.append(st)
            return parts

        def agg_layer(table, layer):
            for g in range(ngroups):
                st0p = subgather(table[0:nhalf, :], idx0_sb, g, K0, "st0", 0)
                st1p = subgather(table[nhalf:npad, :], idx1_sb, g, K1, "st1", 2)
                st0 = lambda c: st0p[c // SUB][:, c % SUB, :]
                st1 = lambda c: st1p[c // SUB][:, c % SUB, :]
                for bb in range(G):
                    b = g * G + bb
                    sl = slice(b * P, (b + 1) * P)
                    oh = ohp.tile([P, KT, P], dt.float16)
                    nc.vector.tensor_tensor(
                        out=oh[:, :, :],
                        in0=dloc_sb[:, b * KT : (b + 1) * KT].to_broadcast([P, KT, P]),
                        in1=iota[:, None, :].to_broadcast([P, KT, P]),
                        op=OP.is_equal,
                    )
                    agg_ps = psum_agg.tile([P, P], dt.float32)
                    for t in range(KT):
                        msg = (
                            st0(bb * K0 + t)
                            if t < K0
                            else st1(bb * K1 + (t - K0))
                        )
                        nc.tensor.matmul(
                            agg_ps[:], lhsT=msg, rhs=oh[:, t, :],
                            start=(t == 0), stop=(t == KT - 1),
                        )
                    aggT = aggp.tile([P, P], dt.float16, tag="aggT")
                    nc.vector.tensor_tensor(
                        out=aggT[:], in0=agg_ps[:], in1=invd_sb[:, sl], op=OP.mult
                    )
                    if layer == 1:
                        o_ps = psum_mm.tile([P, P], dt.float32, tag="mm")
                        nc.tensor.matmul(o_ps[:], lhsT=wsb["Wl1T"][:], rhs=aggT[:], start=True, stop=False)
                        nc.tensor.matmul(o_ps[:], lhsT=wsb["Wr1T"][:], rhs=xT_sb[:, sl], start=False, stop=True)
                        nc.scalar.activation(h1T_sb[:, sl], o_ps[:], AF.Relu, bias=bl1c[:], scale=1.0)
                        # fused layer-2 projection of this block
                        proj_block(h1T_sb, sl, wsb["Wp2T"], bp2b, h2own)
                    else:
                        o_ps = psum_mm.tile([P, D], dt.float32, tag="mm")
                        nc.tensor.matmul(o_ps[:], lhsT=aggT[:], rhs=wsb["Wl2T"][:], start=True, stop=False)
                        nc.tensor.matmul(o_ps[:], lhsT=h1T_sb[:, sl], rhs=wsb["Wr2T"][:], start=False, stop=True)
                        ob = outp.tile([P, D], dt.float32, tag="ob")
                        nc.vector.tensor_tensor(out=ob[:], in0=o_ps[:], in1=bl2b[:], op=OP.add)
                        nc.sync.dma_start(out_own[sl, :], ob[:])

        # ---------------- Phase B: layer-1 aggregate + update + proj2 ------
        agg_layer(table1, 1)

        if DEBUG_OUTPUTS:
            nc.sync.dma_start(dbg1[:, :], h1own[:, :])
            nc.sync.dma_start(dbg2[:, :], h2own[:, :])

        nc.gpsimd.collective_compute(
            "AllGather", OP.bypass, replica_groups=groups_all,
            ins=[h2own[:, :]], outs=[table2[:, :]],
        )

        # ---------------- Phase D: layer-2 aggregate -> out ----------------
        agg_layer(table2, 2)

    nc.compile()
    return nc


def make_in_maps(inputs, per_core, n_nodes, cores):
    nloc, nb, nloc_pad, npad, nhalf = _plan(n_nodes, cores)
    x = np.asarray(inputs["x"], dtype=np.float32)
    consts = dict(
        Wp1T=np.asarray(inputs["Wp1"]).T.astype(np.float16),
        Wl1T=np.asarray(inputs["Wl1"]).T.astype(np.float16),
        Wr1T=np.asarray(inputs["Wr1"]).T.astype(np.float16),
        Wp2T=np.asarray(inputs["Wp2"]).T.astype(np.float16),
        Wl2T=np.asarray(inputs["Wl2"]).T.astype(np.float16),
        Wr2T=np.asarray(inputs["Wr2"]).T.astype(np.float16),
        bp1b=np.broadcast_to(np.asarray(inputs["bp1"], np.float32)[None, :], (P, D)).copy(),
        bl1c=np.asarray(inputs["bl1"], np.float32).reshape(P, 1).copy(),
        bp2b=np.broadcast_to(np.asarray(inputs["bp2"], np.float32)[None, :], (P, D)).copy(),
        bl2b=np.broadcast_to(np.asarray(inputs["bl2"], np.float32)[None, :], (P, D)).copy(),
        iota=np.broadcast_to(np.arange(P, dtype=np.float16)[None, :], (P, P)).copy(),
    )
    in_maps = []
    for c in range(cores):
        xo = np.zeros((nloc_pad, D), dtype=np.float32)
        xo[:nloc] = x[c * nloc : (c + 1) * nloc]
        m = dict(consts)
        m["xT"] = np.ascontiguousarray(xo.T.astype(np.float16))
        m.update(per_core[c])
        in_maps.append(m)
    return in_maps


_BUILT = {}


def _run(inputs, n_nodes, n_edges, cores, G=7, trace=False):
    per_core, K0, K1 = preprocess(inputs["edge_index"], n_nodes, cores)
    key = (n_nodes, cores, K0, K1, G)
    if key not in _BUILT:
        _BUILT[key] = build_nc(n_nodes, cores, K0, K1, G)
    nc = _BUILT[key]
    in_maps = make_in_maps(inputs, per_core, n_nodes, cores)
    res = run_bass_kernel_spmd(nc, in_maps, list(range(cores)), trace=trace)
    nloc, nb, nloc_pad, npad, nhalf = _plan(n_nodes, cores)
    out = np.concatenate([res.results[c]["out_own"][:nloc] for c in range(cores)], axis=0)
    return out.astype(np.float32), res


def kernel(**inputs):
    out, _ = _run(inputs, N_NODES, N_EDGES, CORES, G=7)
    return out
